# revision 1
# baseline (speedup 1.0000x reference)
"""MoE FFN (64 routed experts top-4 + 2 shared experts) on 8 Trainium2 cores.

Strategy (expert parallelism, hardcoded for B=2,T=1024,D=2048,E=64,K=4,H=512):
  - 8 experts per core; router replicated on every core with per-core
    column-permuted Wr so that each core's local experts are columns 0..7.
  - Router matmul in fp32-equivalent precision via hi/lo bf16 splitting
    (3 terms) so that top-4 selection matches the fp32 reference.
  - Top-4 via DVE max8 + match_replace; gate weights from masked exp.
  - Dispatch positions via cumsum-by-matmul (strict lower-triangular ones).
  - Metadata compaction via dma_scatter_add into an HBM slot table;
    per-expert token gather via dma_gather (transposed, bf16).
  - Expert FFN (SwiGLU) in bf16 with fp32 PSUM accumulation, 256-slot
    static capacity (actual max load is ~162 of mean 128).
  - Combine: gate-scaled rows scatter-added into a [N*4, D] bf16 buffer
    at (token*4 + k-rank) -> globally unique slots, no write races.
  - Shared experts tensor-sharded over hidden dim (256 of 2048 per core),
    partial outputs in fp32.
  - ReduceScatter (add) over the 8 cores for both buffers; each core
    emits its 256-token output chunk; host concatenates.
"""

import numpy as np
import ml_dtypes

import concourse.bass as bass
import concourse.mybir as mybir
import concourse.tile as tile
from concourse.tile import add_dep_helper
from concourse import bacc
from concourse.bass import ts
from concourse.bass_utils import run_bass_kernel_spmd
from concourse.masks import make_identity

BF16 = mybir.dt.bfloat16
F32 = mybir.dt.float32
I16 = mybir.dt.int16
I32 = mybir.dt.int32

B, T, D = 2, 1024, 2048
N = B * T                  # 2048 tokens
E, K, H = 64, 4, 512
NCORE = 8
ELOC = E // NCORE          # 8 local experts
CAP = 256                  # static per-expert capacity (max observed load 162)
CAPC = CAP // 128          # slot chunks of 128
TOKC = N // NCORE          # 256 output tokens per core
HSLOC = 2048 // NCORE      # shared hidden per expert per core (256)
NT = N // 128              # 16 token tiles
DC = D // 128              # 16 contraction chunks
TRASH_E = ELOC * CAP       # dispatch trash slot (2048)
META_ROWS = 17 * 128       # 2176 (>= TRASH_E+1, multiple of 128)
TRASH_C = N * K            # combine trash row (8192)
P4_ROWS = 66 * 128         # 8448 (>= TRASH_C+1, multiple of 128)

Ax = mybir.AxisListType
Alu = mybir.AluOpType
Act = mybir.ActivationFunctionType

_CACHE = {}
DEBUG = False


def build_nc():
    nc = bacc.Bacc("TRN2", target_bir_lowering=False, debug=False,
                   num_devices=NCORE)

    # ---- I/O ----
    x_in = nc.dram_tensor("x", [N, D], F32, kind="ExternalInput")
    wr_hi = nc.dram_tensor("wr_hi", [128, DC, E], BF16, kind="ExternalInput")
    wr_lo = nc.dram_tensor("wr_lo", [128, DC, E], BF16, kind="ExternalInput")
    rbias = nc.dram_tensor("rbias", [1, E], F32, kind="ExternalInput")
    wg_in = nc.dram_tensor("wg", [ELOC, 128, DC, H], BF16, kind="ExternalInput")
    wu_in = nc.dram_tensor("wu", [ELOC, 128, DC, H], BF16, kind="ExternalInput")
    wd_in = nc.dram_tensor("wd", [ELOC, 128, H // 128, D], BF16, kind="ExternalInput")
    sg_in = nc.dram_tensor("sg", [128, 4, DC, 128], BF16, kind="ExternalInput")
    su_in = nc.dram_tensor("su", [128, 4, DC, 128], BF16, kind="ExternalInput")
    sd_in = nc.dram_tensor("sd", [128, 4, D], BF16, kind="ExternalInput")
    iden_idx_in = nc.dram_tensor("iden_idx", [128, 128], I16, kind="ExternalInput")
    tick_in = nc.dram_tensor("tick", [128, 128], F32, kind="ExternalInput")
    out_chunk = nc.dram_tensor("out_chunk", [TOKC, D], F32, kind="ExternalOutput")
    tock_out = nc.dram_tensor("tock", [128, 128], F32, kind="ExternalOutput")

    # ---- internal DRAM ----
    xhi_d = nc.dram_tensor("xhi_d", [N, D], BF16)
    idxf_d = nc.dram_tensor("idxf_d", [N, ELOC], F32)       # slot_e per candidate
    meta_d = nc.dram_tensor("meta_d", [META_ROWS, 64], F32)  # per-slot metadata
    p4_d = nc.dram_tensor("p4_d", [P4_ROWS, D], BF16)       # combine buffer
    shared_d = nc.dram_tensor("shared_d", [N, D], BF16)      # shared partials
    rs4_d = nc.dram_tensor("rs4_d", [N * K // NCORE, D], BF16)
    rssh_d = nc.dram_tensor("rssh_d", [TOKC, D], BF16)
    if DEBUG:
        dbg_logT = nc.dram_tensor("dbg_logT", [64, N], F32, kind="ExternalOutput")
        dbg_mall = nc.dram_tensor("dbg_mall", [128, NT, ELOC], F32, kind="ExternalOutput")
        dbg_gl = nc.dram_tensor("dbg_gl", [128, NT, ELOC], F32, kind="ExternalOutput")
        dbg_sc = nc.dram_tensor("dbg_sc", [128, NT, ELOC], F32, kind="ExternalOutput")
        dbg_idxf = nc.dram_tensor("dbg_idxf", [N, ELOC], F32, kind="ExternalOutput")
        dbg_meta = nc.dram_tensor("dbg_meta", [META_ROWS, 4], F32, kind="ExternalOutput")
        dbg_xb = nc.dram_tensor("dbg_xb", [128, DC, CAP], BF16, kind="ExternalOutput")
        dbg_p4 = nc.dram_tensor("dbg_p4", [1024, D], BF16, kind="ExternalOutput")
        dbg_sh = nc.dram_tensor("dbg_sh", [128, D], F32, kind="ExternalOutput")
        dbg_xt = nc.dram_tensor("dbg_xt", [128, DC, 128], BF16, kind="ExternalOutput")

    with tile.TileContext(nc) as tc:
        with tc.tile_pool(name="const", bufs=1) as cp:
            # ================= constants (live whole kernel) ============
            tri = cp.tile([128, 128], F32)          # tri[j,i] = 1 if j<i
            nc.gpsimd.memset(tri[:], 0.0)
            nc.gpsimd.affine_select(out=tri[:], in_=tri[:],
                                    compare_op=Alu.is_ge, fill=1.0,
                                    base=0, channel_multiplier=1,
                                    pattern=[[-1, 128]])
            ident = cp.tile([128, 128], F32)
            make_identity(nc, ident[:])
            ones_col = cp.tile([128, 1], F32)       # lhsT for column sums
            nc.vector.memset(ones_col[:], 1.0)
            ones_row = cp.tile([1, 512], F32)       # rhs for bias broadcast
            nc.vector.memset(ones_row[:], 1.0)
            capoff_i = cp.tile([128, ELOC], I32)    # e*CAP per column
            nc.gpsimd.iota(capoff_i[:], pattern=[[CAP, ELOC]], base=0,
                           channel_multiplier=0)
            capoff = cp.tile([128, ELOC], F32)
            nc.vector.tensor_copy(capoff[:], capoff_i[:])
            trashe_c = cp.tile([128, ELOC], F32)
            nc.vector.memset(trashe_c[:], float(TRASH_E))
            trashc_c = cp.tile([128, ELOC], F32)
            nc.vector.memset(trashc_c[:], float(TRASH_C))
            tokid_i = cp.tile([128, NT], I32)       # p + 128*tt
            nc.gpsimd.iota(tokid_i[:], pattern=[[128, NT]], base=0,
                           channel_multiplier=1)
            tokid_f = cp.tile([128, NT], F32)
            nc.vector.tensor_copy(tokid_f[:], tokid_i[:])
            iden_idx = cp.tile([128, 128], I16)
            nc.sync.dma_start(out=iden_idx[:], in_=iden_idx_in[:])
            tick_t = cp.tile([128, 128], F32)
            nc.sync.dma_start(out=tick_t[:], in_=tick_in[:])
            nc.sync.dma_start(out=tock_out[:], in_=tick_t[:])
            # router-derived tables (filled below, used through FFN)
            gl_all = cp.tile([128, NT, ELOC], F32)   # local gate weights
            m_all = cp.tile([128, NT, ELOC], F32)    # local selection mask
            sc_all = cp.tile([128, NT, ELOC], F32)   # combine slot ids
            cs = cp.tile([1, NT * ELOC], F32)
            carry = cp.tile([1, NT * ELOC], F32)
            disp_idx = cp.tile([128, ELOC * CAP // 16], I16)
            comb_idx = cp.tile([128, ELOC * CAP // 16], I16)
            gmeta = cp.tile([128, ELOC, CAPC], F32)
            tok4 = cp.tile([128, 1], F32)
            nc.vector.tensor_scalar_mul(tok4[:], tokid_f[:, 0:1], 4.0)

            # ================= zero-init combine buffer =================
            with tc.tile_pool(name="zpool", bufs=1) as zp:
                zchunk = 6 * D
                zt = zp.tile([128, zchunk], BF16)
                nc.vector.memset(zt[:], 0.0)
                p4_flat = p4_d[:].rearrange("(p a) c -> p (a c)", p=128)
                for i in range(11):
                    nc.sync.dma_start(out=p4_flat[:, ts(i, zchunk)], in_=zt[:])
                zmeta = zp.tile([128, 17 * 64], F32)
                nc.vector.memset(zmeta[:], 0.0)
                for j in range(17):
                    nc.vector.memset(zmeta[:, j * 64 + 2:j * 64 + 3],
                                     float(TRASH_C))
                nc.sync.dma_start(
                    out=meta_d[:].rearrange("(p j) c -> p (j c)", p=128),
                    in_=zmeta[:])

            # ============ x staging: cast + PE-transpose pipeline ========
            with tc.tile_pool(name="xthi", bufs=1) as xhp:
              xThi = xhp.tile([128, DC, N], BF16)
              with tc.tile_pool(name="xtlo", bufs=1) as xlp:
                xTlo = xlp.tile([128, DC, N], BF16)
                with (
                    tc.tile_pool(name="xstage", bufs=3) as xsp,
                    tc.tile_pool(name="xtpsum", bufs=4, space="PSUM") as xpp,
                ):
                    for tt in range(NT):
                        xt = xsp.tile([128, D], F32, tag="xstage")
                        nc.sync.dma_start(out=xt[:], in_=x_in[ts(tt, 128), :])
                        hi_bf = xsp.tile([128, D], BF16, tag="hib")
                        nc.vector.tensor_copy(hi_bf[:], xt[:])
                        nc.sync.dma_start(out=xhi_d[ts(tt, 128), :],
                                          in_=hi_bf[:])
                        for dc in range(DC):
                            tps = xpp.tile([128, 128], F32, tag="tps",
                                           space="PSUM")
                            nc.tensor.transpose(tps[:], xt[:, ts(dc, 128)],
                                                ident[:])
                            nc.vector.tensor_copy(
                                xThi[:, dc, ts(tt, 128)], tps[:])
                            hf = xsp.tile([128, 128], F32, tag="hft")
                            nc.vector.tensor_copy(
                                hf[:], xThi[:, dc, ts(tt, 128)])
                            nc.vector.tensor_sub(
                                xTlo[:, dc, ts(tt, 128)], tps[:], hf[:])
                if DEBUG:
                    nc.sync.dma_start(out=dbg_xt[:], in_=xThi[:, :, 0:128])

                # ================= router (fp32 via hi/lo bf16) ==========
                with (
                    tc.tile_pool(name="rout", bufs=1) as rp,
                    tc.tile_pool(name="rscr", bufs=2) as sp,
                    tc.tile_pool(name="rpsum", bufs=2, space="PSUM") as psp,
                ):
                    wrh = rp.tile([128, DC, E], BF16)
                    nc.sync.dma_start(out=wrh[:], in_=wr_hi[:])
                    wrl = rp.tile([128, DC, E], BF16)
                    nc.sync.dma_start(out=wrl[:], in_=wr_lo[:])
                    rb = rp.tile([1, E], F32)
                    nc.sync.dma_start(out=rb[:], in_=rbias[:])

                    logitsT = rp.tile([64, N], F32)
                    for tp in range(4):
                        lps = psp.tile([64, 512], F32, tag="routps",
                                       space="PSUM")
                        first = True
                        for dc in range(DC):
                            for (wt, xt_) in ((wrh, xThi), (wrh, xTlo),
                                              (wrl, xThi)):
                                nc.tensor.matmul(lps[:], wt[:, dc, :],
                                                 xt_[:, dc, ts(tp, 512)],
                                                 start=first, stop=False)
                                first = False
                        nc.tensor.matmul(lps[:], rb[:], ones_row[:],
                                         start=False, stop=True)
                        nc.vector.tensor_copy(logitsT[:, ts(tp, 512)], lps[:])
                    if DEBUG:
                        nc.sync.dma_start(out=dbg_logT[:], in_=logitsT[:])

                    # ---- transpose logits; top-4; gates; k-ranks ----
                    for tt in range(NT):
                        ltp = psp.tile([128, 64], F32, tag="ltps",
                                       space="PSUM")
                        nc.tensor.transpose(ltp[:], logitsT[:, ts(tt, 128)],
                                            ident[:64, :64])
                        L = sp.tile([128, 64], F32, tag="logits")
                        nc.vector.tensor_copy(L[:], ltp[:])

                        m8 = sp.tile([128, 8], F32, tag="m8")
                        nc.vector.max(m8[:], L[:])
                        nrm = sp.tile([128, 1], F32, tag="nrm")
                        nc.vector.tensor_scalar_mul(nrm[:], m8[:, 0:1], -1.0)
                        nc.vector.memset(m8[:, 4:8], 3.0e38)
                        Lz = sp.tile([128, 64], F32, tag="lz")
                        nc.vector.match_replace(out=Lz[:], in_to_replace=m8[:],
                                                in_values=L[:],
                                                imm_value=-3.0e38)
                        sel = sp.tile([128, 64], F32, tag="sel")
                        nc.vector.tensor_tensor(out=sel[:], in0=L[:],
                                                in1=Lz[:], op=Alu.not_equal)
                        eexp = sp.tile([128, 64], F32, tag="eexp")
                        nc.scalar.activation(eexp[:], L[:], Act.Exp,
                                             bias=nrm[:, 0:1], scale=1.0)
                        graw = sp.tile([128, 64], F32, tag="graw")
                        nc.vector.tensor_mul(graw[:], eexp[:], sel[:])
                        den = sp.tile([128, 1], F32, tag="den")
                        nc.vector.reduce_sum(den[:], graw[:], axis=Ax.X)
                        rden = sp.tile([128, 1], F32, tag="rden")
                        nc.vector.reciprocal(rden[:], den[:])
                        nc.vector.tensor_scalar_mul(gl_all[:, tt, :],
                                                    graw[:, :ELOC],
                                                    rden[:, 0:1])
                        nc.vector.tensor_copy(m_all[:, tt, :], sel[:, :ELOC])

                        # k-rank among the token's top-4 (by score order)
                        kr = sp.tile([128, ELOC], F32, tag="kr")
                        c0 = sp.tile([128, ELOC], F32, tag="cj")
                        nc.vector.tensor_scalar(kr[:], L[:, :ELOC],
                                                m8[:, 0:1], None,
                                                op0=Alu.is_lt)
                        for j in range(1, 4):
                            nc.vector.tensor_scalar(c0[:], L[:, :ELOC],
                                                    m8[:, j:j + 1], None,
                                                    op0=Alu.is_lt)
                            nc.vector.tensor_add(kr[:], kr[:], c0[:])
                        t4t = sp.tile([128, 1], F32, tag="t4t")
                        nc.vector.tensor_scalar_add(t4t[:], tok4[:],
                                                    float(512 * tt))
                        sct = sp.tile([128, ELOC], F32, tag="sct")
                        nc.vector.tensor_scalar_add(sct[:], kr[:],
                                                    t4t[:, 0:1])
                        # payload carries slot_c - TRASH_C; meta col2 is
                        # initialized to TRASH_C, so unwritten (empty) slots
                        # resolve to the trash row instead of row 0 (whose
                        # racy zero-adds can wipe a real contribution).
                        nc.vector.tensor_scalar_add(sct[:], sct[:],
                                                    -float(TRASH_C))
                        nc.vector.tensor_mul(sc_all[:, tt, :], sct[:],
                                             m_all[:, tt, :])

                    # ---- positions: cumsum over tokens ----
                    csp = psp.tile([1, 128], F32, tag="csps", space="PSUM")
                    nc.tensor.matmul(csp[:], ones_col[:],
                                     m_all[:].rearrange("p t e -> p (t e)"),
                                     start=True, stop=True)
                    nc.vector.tensor_copy(cs[:], csp[:])
                    nc.vector.memset(carry[:, 0:ELOC], 0.0)
                    for t in range(1, NT):
                        nc.vector.tensor_add(carry[:, ts(t, ELOC)],
                                             carry[:, ts(t - 1, ELOC)],
                                             cs[:, ts(t - 1, ELOC)])

                    for tt in range(NT):
                        ppos = psp.tile([128, ELOC], F32, tag="pps",
                                        space="PSUM")
                        nc.tensor.matmul(ppos[:], tri[:], m_all[:, tt, :],
                                         start=True, stop=False)
                        nc.tensor.matmul(ppos[:], ones_row[:, 0:128],
                                         carry[:, ts(tt, ELOC)],
                                         start=False, stop=True)
                        pos = sp.tile([128, ELOC], F32, tag="pos")
                        nc.vector.tensor_copy(pos[:], ppos[:])
                        okc = sp.tile([128, ELOC], F32, tag="okc")
                        nc.vector.tensor_scalar(okc[:], pos[:], float(CAP),
                                                None, op0=Alu.is_lt)
                        nc.vector.tensor_mul(okc[:], okc[:], m_all[:, tt, :])
                        se = sp.tile([128, ELOC], F32, tag="se")
                        nc.vector.tensor_add(se[:], pos[:], capoff[:])
                        sesel = sp.tile([128, ELOC], F32, tag="sesel")
                        nc.vector.tensor_scalar_add(se[:], se[:],
                                                    -float(TRASH_E))
                        nc.vector.tensor_mul(se[:], se[:], okc[:])
                        nc.vector.tensor_scalar_add(sesel[:], se[:],
                                                    float(TRASH_E))
                        nc.sync.dma_start(
                            out=idxf_d[:].rearrange("(t p) e -> p t e",
                                                    p=128)[:, tt, :],
                            in_=sesel[:])

                if DEBUG:
                    nc.sync.dma_start(out=dbg_mall[:], in_=m_all[:])
                    nc.sync.dma_start(out=dbg_gl[:], in_=gl_all[:])
                    nc.sync.dma_start(out=dbg_sc[:], in_=sc_all[:])
                    nc.sync.dma_start(out=dbg_idxf[:], in_=idxf_d[:])

              # ============ metadata compaction scatter ============
              with tc.tile_pool(name="meta", bufs=1) as mp:
                  scat_f = mp.tile([128, ELOC * 128], F32)
                  src16 = idxf_d[:].rearrange("(j p) e -> p e j", p=16)
                  for r in range(8):
                      nc.sync.dma_start(
                          out=scat_f[ts(r, 16), :].rearrange(
                              "p (e j) -> p e j", e=ELOC),
                          in_=src16)
                  scat_idx = mp.tile([128, ELOC * 128], I16)
                  nc.vector.tensor_copy(scat_idx[:], scat_f[:])

                  payload = mp.tile([128, ELOC * NT, 64], F32)
                  nc.gpsimd.memset(payload[:], 0.0)
                  for e in range(ELOC):
                      nc.vector.tensor_copy(
                          payload[:, ts(e, NT), 0:1].rearrange(
                              "p t o -> p (t o)"),
                          tokid_f[:])
                  nc.vector.tensor_copy(
                      payload[:, :, 1:2].rearrange("p (e t) o -> p e (t o)",
                                                   e=ELOC),
                      gl_all[:].rearrange("p t e -> p e t"))
                  nc.vector.tensor_copy(
                      payload[:, :, 2:3].rearrange("p (e t) o -> p e (t o)",
                                                   e=ELOC),
                      sc_all[:].rearrange("p t e -> p e t"))
                  # split into 4 calls: descriptor carveout is 1024 descs
                  for c in range(4):
                      nc.gpsimd.dma_scatter_add(
                          meta_d[:, :],
                          payload[:, ts(c, ELOC * NT // 4), :],
                          scat_idx[:, ts(c, ELOC * 128 // 4)],
                          N * ELOC // 4, N * ELOC // 4, 64)

                  # ---- extract per-expert tables ----
                  dispf = mp.tile([128, ELOC * CAP // 16], F32)
                  combf = mp.tile([128, ELOC * CAP // 16], F32)
                  dsrc = meta_d[:ELOC * CAP, 0:1].rearrange(
                      "(e j p) o -> p (e j o)", p=16, j=CAP // 16)
                  csrc = meta_d[:ELOC * CAP, 2:3].rearrange(
                      "(e j p) o -> p (e j o)", p=16, j=CAP // 16)
                  for r in range(8):
                      nc.sync.dma_start(out=dispf[ts(r, 16), :], in_=dsrc)
                      nc.sync.dma_start(out=combf[ts(r, 16), :], in_=csrc)
                  nc.vector.tensor_copy(disp_idx[:], dispf[:])
                  nc.vector.tensor_copy(comb_idx[:], combf[:])
                  nc.sync.dma_start(
                      out=gmeta[:],
                      in_=meta_d[:ELOC * CAP, 1:2].rearrange(
                          "(e c p) o -> p e (c o)", p=128, c=CAPC))

              if DEBUG:
                  nc.sync.dma_start(out=dbg_meta[:], in_=meta_d[:, 0:4])

              # ================= shared experts =================
              with (
                  tc.tile_pool(name="shw", bufs=1) as shp,
                  tc.tile_pool(name="shs", bufs=2) as ssp,
                  tc.tile_pool(name="shpsum", bufs=2, space="PSUM") as spp,
              ):
                  sgw = shp.tile([128, 4, DC, 128], BF16)
                  nc.scalar.dma_start(out=sgw[:], in_=sg_in[:])
                  suw = shp.tile([128, 4, DC, 128], BF16)
                  nc.scalar.dma_start(out=suw[:], in_=su_in[:])
                  sdw = shp.tile([128, 4, D], BF16)
                  nc.scalar.dma_start(out=sdw[:], in_=sd_in[:])

                  hs_bf = shp.tile([128, 4, N], BF16)
                  for hc in range(4):
                      for tp in range(4):
                          gp = spp.tile([128, 512], F32, tag="sgp",
                                        space="PSUM")
                          up = spp.tile([128, 512], F32, tag="sup",
                                        space="PSUM")
                          for dc in range(DC):
                              nc.tensor.matmul(gp[:], sgw[:, hc, dc, :],
                                               xThi[:, dc, ts(tp, 512)],
                                               start=(dc == 0),
                                               stop=(dc == DC - 1))
                          for dc in range(DC):
                              nc.tensor.matmul(up[:], suw[:, hc, dc, :],
                                               xThi[:, dc, ts(tp, 512)],
                                               start=(dc == 0),
                                               stop=(dc == DC - 1))
                          sil = ssp.tile([128, 512], F32, tag="ssil")
                          nc.scalar.activation(sil[:], gp[:], Act.Sigmoid)
                          nc.vector.tensor_mul(sil[:], sil[:], gp[:])
                          nc.vector.tensor_mul(hs_bf[:, hc, ts(tp, 512)],
                                               sil[:], up[:])

                  for tt in range(NT):
                      ysh = ssp.tile([128, D], BF16, tag="ysh")
                      for dco in range(4):
                          yp = spp.tile([128, 512], F32, tag="syp",
                                        space="PSUM")
                          for hc in range(4):
                              nc.tensor.matmul(yp[:],
                                               hs_bf[:, hc, ts(tt, 128)],
                                               sdw[:, hc, ts(dco, 512)],
                                               start=(hc == 0),
                                               stop=(hc == 3))
                          nc.scalar.activation(ysh[:, ts(dco, 512)], yp[:],
                                               Act.Copy)
                      nc.sync.dma_start(out=shared_d[ts(tt, 128), :],
                                        in_=ysh[:])

              # shared collective issued before FFN Pool work so it
              # overlaps the expert compute phase.
              nc.gpsimd.collective_compute(
                  "ReduceScatter", Alu.add,
                  replica_groups=[list(range(NCORE))],
                  ins=[shared_d[:, :]], outs=[rssh_d[:, :]])

            # ================= routed expert FFN =================
            with (
                tc.tile_pool(name="ffnw", bufs=2) as fwp,
                tc.tile_pool(name="ffna", bufs=2) as fap,
                tc.tile_pool(name="ffnp", bufs=2, space="PSUM") as fpp,
            ):
                for e in range(ELOC):
                    wgt = fwp.tile([128, DC, H], BF16, tag="wg")
                    nc.sync.dma_start(out=wgt[:], in_=wg_in[e])
                    wut = fwp.tile([128, DC, H], BF16, tag="wu")
                    nc.scalar.dma_start(out=wut[:], in_=wu_in[e])
                    wdt = fwp.tile([128, H // 128, D], BF16, tag="wd")
                    nc.sync.dma_start(out=wdt[:], in_=wd_in[e])

                    xb = fap.tile([128, DC, CAP], BF16, tag="xb")
                    nc.gpsimd.dma_gather(xb[:], xhi_d[:, :],
                                         disp_idx[:, ts(e, CAP // 16)],
                                         CAP, CAP, D, transpose=True)
                    if DEBUG and e == 0:
                        nc.sync.dma_start(out=dbg_xb[:], in_=xb[:])

                    hbf = fap.tile([128, H // 128, CAP], BF16, tag="hbf")
                    for hc in range(H // 128):
                        gp = fpp.tile([128, CAP], F32, tag="fgp", space="PSUM")
                        up = fpp.tile([128, CAP], F32, tag="fup", space="PSUM")
                        for dc in range(DC):
                            nc.tensor.matmul(gp[:], wgt[:, dc, ts(hc, 128)],
                                             xb[:, dc, :],
                                             start=(dc == 0),
                                             stop=(dc == DC - 1))
                        for dc in range(DC):
                            nc.tensor.matmul(up[:], wut[:, dc, ts(hc, 128)],
                                             xb[:, dc, :],
                                             start=(dc == 0),
                                             stop=(dc == DC - 1))
                        sil = fap.tile([128, CAP], F32, tag="fsil")
                        nc.scalar.activation(sil[:], gp[:], Act.Sigmoid)
                        nc.vector.tensor_mul(sil[:], sil[:], gp[:])
                        nc.vector.tensor_mul(hbf[:, hc, :], sil[:], up[:])

                    y_bf = fap.tile([128, CAPC, D], BF16, tag="ybf")
                    for sc in range(CAPC):
                        for dco in range(4):
                            yp = fpp.tile([128, 512], F32, tag="fyp",
                                          space="PSUM")
                            for hc in range(H // 128):
                                nc.tensor.matmul(yp[:],
                                                 hbf[:, hc, ts(sc, 128)],
                                                 wdt[:, hc, ts(dco, 512)],
                                                 start=(hc == 0),
                                                 stop=(hc == 3))
                            nc.scalar.activation(y_bf[:, sc, ts(dco, 512)],
                                                 yp[:], Act.Copy,
                                                 scale=gmeta[:, e, sc:sc + 1])
                    nc.gpsimd.dma_scatter_add(p4_d[:, :], y_bf[:],
                                              comb_idx[:, ts(e, CAP // 16)],
                                              CAP, CAP, D)

            if DEBUG:
                nc.sync.dma_start(out=dbg_p4[:], in_=p4_d[0:1024, :])
                nc.sync.dma_start(out=dbg_sh[:], in_=shared_d[0:128, :])

            # ================= collectives + finalize =================
            nc.gpsimd.collective_compute(
                "ReduceScatter", Alu.add, replica_groups=[list(range(NCORE))],
                ins=[p4_d[:N * K, :]], outs=[rs4_d[:, :]])

            with tc.tile_pool(name="fin", bufs=2) as fp:
                for tt in range(2):
                    r4 = fp.tile([128, K, D], BF16, tag="r4")
                    nc.sync.dma_start(
                        out=r4[:],
                        in_=rs4_d[:].rearrange("(p k) c -> p k c",
                                               k=K)[ts(tt, 128)])
                    sh = fp.tile([128, D], BF16, tag="sh")
                    nc.sync.dma_start(out=sh[:], in_=rssh_d[ts(tt, 128), :])
                    a01 = fp.tile([128, D], F32, tag="a01")
                    nc.vector.tensor_add(a01[:], r4[:, 0, :], r4[:, 1, :])
                    a23 = fp.tile([128, D], F32, tag="a23")
                    nc.vector.tensor_add(a23[:], r4[:, 2, :], r4[:, 3, :])
                    nc.vector.tensor_add(a01[:], a01[:], a23[:])
                    nc.vector.tensor_add(a01[:], a01[:], sh[:])
                    nc.sync.dma_start(out=out_chunk[ts(tt, 128), :], in_=a01[:])

    nc.compile()
    return nc


def _prep_inputs(x, Wr, router_bias, Wg, Wu, Wd, Sg, Su, Sd):
    """Build the 8 per-core input maps (expert-parallel sharding)."""
    bf = ml_dtypes.bfloat16
    flat = np.ascontiguousarray(np.asarray(x, np.float32).reshape(N, D))
    Wr = np.asarray(Wr, np.float32)
    router_bias = np.asarray(router_bias, np.float32)
    Wg = np.asarray(Wg); Wu = np.asarray(Wu); Wd = np.asarray(Wd)
    Sg = np.asarray(Sg); Su = np.asarray(Su); Sd = np.asarray(Sd)

    base = np.arange(N, dtype=np.int16).reshape(128, 16).T  # [16,128]
    iden_idx = np.ascontiguousarray(np.tile(base, (8, 1)))  # [128,128]

    maps = []
    for m in range(NCORE):
        loc = list(range(m * ELOC, (m + 1) * ELOC))
        rest = [e for e in range(E) if e not in loc]
        perm = loc + rest
        wr_p = Wr[:, perm]
        wr_h = wr_p.astype(bf)
        wr_l = (wr_p - wr_h.astype(np.float32)).astype(bf)
        sg_m = np.empty((4, DC, 128, 128), bf)
        su_m = np.empty((4, DC, 128, 128), bf)
        sd_m = np.empty((4, 128, D), bf)
        for hc in range(4):
            s = hc // 2
            c0 = m * HSLOC + (hc % 2) * 128
            sg_m[hc] = Sg[s].astype(bf).reshape(DC, 128, 2048)[:, :, c0:c0 + 128]
            su_m[hc] = Su[s].astype(bf).reshape(DC, 128, 2048)[:, :, c0:c0 + 128]
            sd_m[hc] = Sd[s, c0:c0 + 128, :].astype(bf)
        maps.append({
            "x": flat,
            "tick": np.zeros((128, 128), np.float32),
            "wr_hi": np.ascontiguousarray(
                wr_h.reshape(DC, 128, E).transpose(1, 0, 2)),
            "wr_lo": np.ascontiguousarray(
                wr_l.reshape(DC, 128, E).transpose(1, 0, 2)),
            "rbias": np.ascontiguousarray(router_bias[perm].reshape(1, E)),
            "wg": np.ascontiguousarray(
                Wg[loc].astype(bf).reshape(ELOC, DC, 128, H).transpose(0, 2, 1, 3)),
            "wu": np.ascontiguousarray(
                Wu[loc].astype(bf).reshape(ELOC, DC, 128, H).transpose(0, 2, 1, 3)),
            "wd": np.ascontiguousarray(
                Wd[loc].astype(bf).reshape(ELOC, H // 128, 128, D).transpose(0, 2, 1, 3)),
            "sg": np.ascontiguousarray(sg_m.transpose(2, 0, 1, 3)),
            "su": np.ascontiguousarray(su_m.transpose(2, 0, 1, 3)),
            "sd": np.ascontiguousarray(sd_m.transpose(1, 0, 2)),
            "iden_idx": iden_idx,
        })
    return maps


def kernel(**inputs):
    if "nc" not in _CACHE:
        _CACHE["nc"] = build_nc()
    nc = _CACHE["nc"]
    maps = _prep_inputs(**inputs)
    res = run_bass_kernel_spmd(nc, maps, core_ids=list(range(NCORE)))
    out = np.concatenate([res.results[i]["out_chunk"] for i in range(NCORE)], 0)
    return out.reshape(B, T, D)

